# revision 53
# baseline (speedup 1.0000x reference)
"""Longformer self-attention Trainium2 kernel (8-core SPMD), v3.

Sharding: core c handles batch b = c//4 and heads [3*(c%4), 3*(c%4)+3).
Each core computes its 3 heads' [4096, 64] outputs; the host divides by
the softmax denominator (exported as an extra row) and reassembles
[2, 4096, 768].

v3 design notes (on top of v2's fused projection/attention pipeline):
  - PE HAM clock gate: the PE runs at 1.2 GHz unless a ~3.4us activity
    window is busy; small-N matmul stretches re-throttle it.  v3 keeps
    every hot-loop matmul at N>=256:
      * PV is computed transposed: stationary v-chunk [128,65] (64 dims
        + ones column), moving bexp [128 kpos, 256 queries] -> attnT
        [65, 256] in PSUM, one matmul per kpos chunk (no half split).
        Row 64 is the softmax denominator; normalization happens on the
        HOST after the f32 attnT/denominator tile is DMA'd out.
      * QK (K=64 contraction) issues as concurrent row-tile pairs:
        heads 0/1 are packed at partitions 0-63/64-127 of shared q/k
        tiles, head 2 is duplicated into both halves via SBUF DMA and
        pairs its own chunks.  tile_position=(0,0)/(64,0); the pair's
        PSUM outputs come from disjoint even/odd pools (bank safety).
      * global-row PV col-tiles 3 heads at tile_position (0,0/32/64).
  - Weight columns are reordered per-core so each 128-wide PSUM
    d-chunk evacuates with a single DVE add: [q0|q1][k0|k1][q2|k2]
    [kg0|kg1][kg2].  Biases ride along as per-partition columns.
  - Masks: multiplicative 0/1 bf16 masks after exp (packed classes).
  - exp() without max subtraction (logits are O(1), safe in f32).
"""

import os
import sys

sys.path.insert(0, "/opt/trn_rl_repo")

import numpy as np
import ml_dtypes

B, S, Dm, H, WIN, G, HD = 2, 4096, 768, 12, 256, 16, 64
HPC = 3            # heads per core
NCORES = 8
DPC = HPC * HD     # 192 output dims per core
NB = S // WIN      # 16 query blocks
NKC = S // 128     # 32 kpos chunks of 128
NST = 8            # s-tiles of 512
SCALE = 1.0 / 8.0  # 1/sqrt(64)
QKK = 3 * DPC      # 576 merged q|k|kg output cols

_CACHE = {}


def _chunk_range(t):
    if t == 0:
        return 2, 6
    if t == NB - 1:
        return 0, 4
    return 0, 6


def _build_mask(t, c):
    p = np.arange(128)[:, None]
    r = np.arange(256)[None, :]
    kpos = (2 * t - 2 + c) * 128 + p
    i = 256 * t + r
    keep = (np.abs(kpos - i) <= WIN) & (kpos >= 0) & (kpos < S) & (kpos >= G)
    return keep.astype(np.float32)


def _mask_classes():
    """Packed mask chunks [nmask, 128, 256] and, per class, the list of
    (chunk_lo, chunk_hi, packed_idx) multiply ops."""
    packed = []

    def cls_of(t):
        return 0 if t == 0 else (1 if t == 1 else (3 if t == NB - 1 else 2))

    ops = {}
    for cls, trep in ((0, 0), (1, 1), (2, 7), (3, NB - 1)):
        cl, ch = _chunk_range(trep)
        need = [
            c for c in range(cl, ch) if not np.all(_build_mask(trep, c) == 1.0)
        ]
        groups = []
        for c in need:
            if groups and groups[-1][-1] == c - 1:
                groups[-1].append(c)
            else:
                groups.append([c])
        clsops = []
        for grp in groups:
            base = len(packed)
            for c in grp:
                packed.append(_build_mask(trep, c))
            clsops.append((grp[0], grp[-1] + 1, base))
        ops[cls] = clsops
    for t in range(NB):
        cls = cls_of(t)
        cl, ch = _chunk_range(t)
        covered = set()
        for lo, hi, base in ops[cls]:
            for c in range(lo, hi):
                assert np.array_equal(packed[base + (c - lo)], _build_mask(t, c))
                covered.add(c)
        for c in range(cl, ch):
            if c not in covered:
                assert np.all(_build_mask(t, c) == 1.0), (t, c)
    return np.stack(packed), ops, cls_of


def _patch_drain_and_barrier():
    """The walrus build in this container rejects >1 sync-wait on the CTRL
    (Drain) instruction that TileContext emits at exit.  Split the waits:
    keep one on the drain, emit the rest as explicit single-sem wait_ge
    instructions on the sync engine before the barrier."""
    import concourse.tile as tile
    from concourse import mybir
    from concourse.vector_clock import ScopedClock

    if getattr(tile.TileContext, "_ant_drain_patch", False):
        return

    def _drain_and_barrier(self, tick_clock, wait_clock):
        nc = self.nc
        drain_inst = nc.sync.drain()
        wait_clock.add_sem_waits(
            drain_inst.ins, ScopedClock({None: tick_clock.global_clock})
        )
        si = drain_inst.ins.sync_info
        waits = list(si.on_wait) if si is not None else []
        if len(waits) > 1:
            drain_inst.ins.sync_info = mybir.SyncInfo(
                on_wait=[waits[0]], on_update=list(si.on_update)
            )
            allocated = self.sems.allocated()
            by_name = {}
            for key, sem in allocated.items():
                by_name[str(key)] = sem
                nm = getattr(sem, "name", None)
                if nm is not None:
                    by_name[str(nm)] = sem
            for w in waits[1:]:
                sem = by_name[w.ant_name]
                nc.sync.wait_ge(sem, w.wait_value)
        nc.all_engine_barrier()
        assert self.sems is not None
        popped = nc._tile_sem_poison_stack.pop()
        assert popped is self._sem_poison
        nc.clear_and_free_semaphores(list(self.sems.allocated().values()))
        nc.all_engine_barrier()

    tile.TileContext._drain_and_barrier = _drain_and_barrier
    tile.TileContext._ant_drain_patch = True


def _build_program():
    import concourse.bass as bass
    import concourse.tile as tile
    from concourse import bacc, mybir

    _patch_drain_and_barrier()

    f32 = mybir.dt.float32
    bf16 = mybir.dt.bfloat16
    AF = mybir.ActivationFunctionType
    AFexp = AF.Exp

    nc = bacc.Bacc(None)

    xT = nc.dram_tensor("xT", [128, 6, S], bf16, kind="ExternalInput")
    # merged+reordered weight: [q0|q1][k0|k1][q2|k2][kg0|kg1][kg2]
    Wqkk = nc.dram_tensor("Wqkk", [128, 6, QKK], bf16, kind="ExternalInput")
    # qg weight reordered [qg0|qg1][qg2]
    Wqg = nc.dram_tensor("Wqg", [128, 6, DPC], bf16, kind="ExternalInput")
    Wvvg = nc.dram_tensor("Wvvg", [128, 6, 2 * DPC], bf16, kind="ExternalInput")
    # bias columns [128, 8]: col dc<5 = qkk evac bias for that d-chunk
    # (stacked per 64-row half); col 5 rows 0-63 = bk_h2; col 6 = qg01
    # stacked; col 7 rows 0-63 = bqg_h2
    b_all = nc.dram_tensor("b_all", [128, 8], f32, kind="ExternalInput")
    b_vvg = nc.dram_tensor("b_vvg", [128, HPC, 2, HD], f32, kind="ExternalInput")
    # unnormalized attn output (row 64 = softmax denominator)
    outT_d = nc.dram_tensor("outT", [HPC, NB, HD + 1, 256], f32, kind="ExternalOutput")
    outG_d = nc.dram_tensor("outG", [96, HD + 1], f32, kind="ExternalOutput")

    packed_np, mask_ops, tcls = _mask_classes()
    NMASK = packed_np.shape[0]
    masks_d = nc.inline_tensor(
        np.ascontiguousarray(packed_np.transpose(1, 0, 2)).astype(
            ml_dtypes.bfloat16
        ),
        name="masks",
    )

    from contextlib import ExitStack

    with tile.TileContext(nc) as tc, ExitStack() as ctx:
        const = ctx.enter_context(tc.tile_pool(name="const", bufs=1))
        ph = ctx.enter_context(tc.tile_pool(name="ph", bufs=1))
        xpool = ctx.enter_context(tc.tile_pool(name="xpool", bufs=2))
        bx = ctx.enter_context(tc.tile_pool(name="bx", bufs=12))
        ob = ctx.enter_context(tc.tile_pool(name="ob", bufs=4))
        # QK score pools: pair partners must land in different PSUM banks,
        # so base-0 tiles come from psAe and base-64 tiles from psAo.
        psAe = ctx.enter_context(tc.tile_pool(name="psAe", bufs=3, space="PSUM"))
        psAo = ctx.enter_context(tc.tile_pool(name="psAo", bufs=3, space="PSUM"))
        psB = ctx.enter_context(tc.tile_pool(name="psB", bufs=2, space="PSUM"))

        # startup loads: SP carries the critical path (wqkk-dc0 + x-tile 0,
        # then remaining weight strips); the Activation DGE carries the rest
        # in parallel (each dma_start costs ~650ns of issue time).
        wqkk = const.tile([128, 6, QKK], bf16, tag="wqkk", name="wqkk")
        xt0 = xpool.tile([128, 6, 512], bf16, tag="xt", name="xt")
        nc.sync.dma_start(out=wqkk[:, :, 0:128], in_=Wqkk[:, :, 0:128])
        nc.sync.dma_start(out=xt0, in_=xT[:, :, 0:512])
        for dcs in range(1, 5):
            e0 = 128 * dcs
            e1 = min(e0 + 128, QKK)
            nc.sync.dma_start(out=wqkk[:, :, e0:e1], in_=Wqkk[:, :, e0:e1])

        wvvg = const.tile([128, 6, 2 * DPC], bf16, tag="wvvg", name="wvvg")
        nc.scalar.dma_start(out=wvvg, in_=Wvvg[:])
        wqg = const.tile([128, 6, DPC], bf16, tag="wqg", name="wqg")
        nc.scalar.dma_start(out=wqg, in_=Wqg[:])
        ball = const.tile([128, 8], f32, tag="ball", name="ball")
        nc.scalar.dma_start(out=ball, in_=b_all[:])
        bvvg_sb = const.tile([128, HPC, 2, HD], f32, tag="bvvg", name="bvvg_sb")
        nc.scalar.dma_start(out=bvvg_sb, in_=b_vvg[:])
        masks_sb = const.tile([128, NMASK, 256], bf16, tag="masks", name="masks_sb")
        nc.scalar.dma_start(out=masks_sb, in_=masks_d[:])

        # PE warm-up: ~5us of dummy matmuls on never-written SBUF keep the
        # HAM activity window busy while the first DMAs land (the PE
        # otherwise starts at 1.2 GHz).  Results go to a scratch PSUM
        # tile that is never read.
        scrA = const.tile([128, 128], bf16, tag="scrA", name="scrA")
        scrB = const.tile([128, 512], bf16, tag="scrB", name="scrB")
        nc.gpsimd.memset(scrA[:], 0.5)
        nc.gpsimd.memset(scrB[:], 0.5)
        WARMUP = os.environ.get("KWARM", "0") == "1"
        if WARMUP:
            for wi in range(2):
                wps = psB.tile([128, 512], f32, tag="small", name="warm")
                for wj in range(12):
                    nc.tensor.matmul(wps, scrA, scrB, start=wj == 0, stop=wj == 11)

        # ---- persistent tensors ----
        # packed transposed projections: [0:64]=head A, [64:128]=head B
        qP = ph.tile([128, S], bf16, tag="qP", name="qP")     # q0|q1
        kP = ph.tile([128, S], bf16, tag="kP", name="kP")     # k0|k1
        kgP = ph.tile([128, S], bf16, tag="kgP", name="kgP")  # kg0|kg1
        q2 = ph.tile([128, S], bf16, tag="q2", name="q2")     # q2|q2(dup)
        k2 = ph.tile([128, S], bf16, tag="k2", name="k2")     # k2|k2(dup)
        kg2 = ph.tile([128, S], bf16, tag="kg2", name="kg2")  # kg2|kg2(dup)
        qgP = ph.tile([128, G], bf16, tag="qgP", name="qgP")  # qg0|qg1
        qg2 = ph.tile([128, G], bf16, tag="qg2", name="qg2")  # qg2|qg2(dup)

        def qhalf(h):  # (tile, part_lo) for q of head h
            return (qP, 0) if h == 0 else ((qP, 64) if h == 1 else (q2, 0))

        def khalf(h):
            return (kP, 0) if h == 0 else ((kP, 64) if h == 1 else (k2, 0))

        # v/vg interleaved with ones column: [:, chunk, 2h+0, :] = v head h,
        # [:, chunk, 2h+1, :] = vg head h ([:, :, :, 64] = 1.0)
        vall = ph.tile([128, NKC, 2 * HPC, HD + 1], bf16, tag="vall", name="vall")
        nc.vector.memset(vall[:, :, :, HD : HD + 1], 1.0)
        selexp = [
            ph.tile([G, S], bf16, tag=f"selexp{h}", name=f"selexp{h}")
            for h in range(HPC)
        ]
        eg3 = ph.tile([128, NKC, HPC, G], bf16, tag="eg3", name="eg3")

        def mm(out, lhsT, rhs, start, stop, tile_position=None):
            nc.tensor.matmul(
                out, lhsT, rhs, start=start, stop=stop, tile_position=tile_position
            )

        # qkk evac destinations per d-chunk: (dst_tile, rows, bias_col_ap)
        def emit_proj(st, xt):
            ssl = slice(512 * st, 512 * (st + 1))
            evac = [
                [(qP, None, 0)],
                [(kP, None, 1)],
                [(q2, (0, 64), 2), (k2, (64, 128), 5)],
                [(kgP, None, 3)],
                [(kg2, (0, 64), 4)],
            ]
            for dc in range(5):
                d0 = 128 * dc
                d1 = min(d0 + 128, QKK)
                ps = psB.tile([d1 - d0, 512], f32, tag="small", name="psqkk")
                for kc in range(6):
                    mm(ps, wqkk[:, kc, d0:d1], xt[:, kc, :], kc == 0, kc == 5)
                for dst, rows, bcol in evac[dc]:
                    if rows is None:
                        nc.vector.tensor_scalar_add(
                            dst[0 : d1 - d0, ssl], ps, ball[0 : d1 - d0, bcol : bcol + 1]
                        )
                    else:
                        r0, r1 = rows
                        nc.vector.tensor_scalar_add(
                            dst[0 : r1 - r0, ssl],
                            ps[r0:r1, :],
                            ball[0 : r1 - r0, bcol : bcol + 1],
                        )
            # duplicate head-2 halves: rows 0-63 -> rows 64-127 (QK pairing)
            for dst in (q2, k2):
                nc.sync.dma_start(out=dst[64:128, ssl], in_=dst[0:64, ssl])

            if st == 0:
                # qg: [128, 16] = qg0|qg1 stacked (full-M matmul), + qg2
                psq = psB.tile([128, G], f32, tag="small", name="psqg")
                for kc in range(6):
                    mm(psq, wqg[:, kc, 0:128], xt[:, kc, 0:G], kc == 0, kc == 5)
                nc.vector.tensor_scalar_add(qgP, psq, ball[:, 6:7])
                psq2 = psB.tile([64, G], f32, tag="small", name="psqg2")
                for kc in range(6):
                    mm(psq2, wqg[:, kc, 128:192], xt[:, kc, 0:G], kc == 0, kc == 5)
                nc.vector.tensor_scalar_add(qg2[0:64, :], psq2, ball[0:64, 7:8])

            # sel = q . k[:G] for this s-tile's queries; heads 0/1 paired
            sel01 = []
            for h in range(2):
                kt, p0 = khalf(h)
                qt, _ = qhalf(h)
                sps = psB.tile([G, 512], f32, tag="small", name="sps")
                mm(
                    sps,
                    kt[p0 : p0 + 64, 0:G],
                    qt[p0 : p0 + 64, ssl],
                    True,
                    True,
                    tile_position=(p0, 0),
                )
                sel01.append(sps)
            for h in range(2):
                nc.scalar.activation(
                    out=selexp[h][:, ssl], in_=sel01[h], func=AFexp
                )
            sps2 = psB.tile([G, 512], f32, tag="small", name="sps")
            mm(sps2, k2[0:64, 0:G], q2[0:64, ssl], True, True)
            nc.scalar.activation(out=selexp[2][:, ssl], in_=sps2, func=AFexp)

            # v/vg groups with the tiny glT matmuls interleaved BETWEEN
            # groups: an isolated cluster of N=16 matmuls starves the PE HAM
            # activity window and re-throttles the clock to 1.2 GHz for the
            # next ~3.4us.  One PSUM tile per glT head (adjacent-head glT
            # MMs use different PE row groups and execute concurrently --
            # their outputs must sit in different PSUM banks).
            gparams = (((kgP, 0), qgP), ((kgP, 64), qgP), ((kg2, 0), qg2))
            for sc in range(4):
                ci = 4 * st + sc
                msl = slice(128 * sc, 128 * (sc + 1))
                psv = psB.tile([128, 2 * DPC], f32, tag="small", name="psv")
                for kc in range(6):
                    mm(psv, xt[:, kc, msl], wvvg[:, kc, :], kc == 0, kc == 5)
                src = bass.AP(
                    tensor=psv.tensor,
                    offset=psv.offset,
                    ap=[psv.ap[0], [HD, HPC], [DPC, 2], [1, HD]],
                )
                dst = vall[:, ci, :, 0:HD].rearrange("p (h g) d -> p h g d", h=HPC)
                nc.vector.tensor_add(dst, src, bvvg_sb)
                if sc < 3:
                    (kgt, p0), qgt = gparams[sc]
                    gps = psB.tile([128, 4, G], f32, tag="small", name="gps")
                    for gsc in range(4):
                        gci = 4 * st + gsc
                        csl = slice(128 * gci, 128 * (gci + 1))
                        mm(
                            gps[:, gsc, :],
                            kgt[p0 : p0 + 64, csl],
                            qgt[p0 : p0 + 64, :],
                            True,
                            True,
                            tile_position=(p0, 0),
                        )
                    nc.scalar.activation(
                        out=eg3[:, 4 * st : 4 * st + 4, sc, :],
                        in_=gps,
                        func=AFexp,
                    )

        def emit_qk(t):
            """Paired QK scores + exp + mask for one block, 3 heads."""
            cl, ch = _chunk_range(t)
            n = ch - cl
            qsl = slice(256 * t, 256 * (t + 1))
            cls = tcls(t)
            bexps = [
                bx.tile([128, 6, 256], bf16, tag="bexp", name="bexp")
                for _ in range(HPC)
            ]

            # heads 0/1 pair on the same chunk (rows 0-63 vs 64-127); head 2
            # pairs chunk cl+i (base 0) with chunk cl+n/2+i (base 64, dup).
            # New [128,2,256] tile every 2 chunks.
            tiles01 = {0: [], 1: []}
            for ci, c in enumerate(range(cl, ch)):
                j = 2 * t - 2 + c
                jsl = slice(128 * j, 128 * (j + 1))
                if ci % 2 == 0:
                    se = psAe.tile([128, 2, 256], f32, tag="se", name="se")
                    so = psAo.tile([128, 2, 256], f32, tag="so", name="so")
                    tiles01[0].append((se, c))
                    tiles01[1].append((so, c))
                slot = ci % 2
                mm(se[:, slot, :], kP[0:64, jsl], qP[0:64, qsl], True, True,
                   tile_position=(0, 0))
                mm(so[:, slot, :], kP[64:128, jsl], qP[64:128, qsl], True, True,
                   tile_position=(64, 0))
                if slot == 1 or ci == n - 1:
                    width = slot + 1
                    for h, tl in ((0, tiles01[0][-1]), (1, tiles01[1][-1])):
                        tile_, c0 = tl
                        nc.scalar.activation(
                            out=bexps[h][:, c0 : c0 + width, :],
                            in_=tile_[:, 0:width, :],
                            func=AFexp,
                        )
            # head 2: pair (cl+i, cl+n/2+i) via the duplicated halves
            half = n // 2
            t2e, t2o = [], []
            for i in range(half):
                ca = cl + i
                cb = cl + half + i
                ja = 2 * t - 2 + ca
                jb = 2 * t - 2 + cb
                if i % 2 == 0:
                    se = psAe.tile([128, 2, 256], f32, tag="se", name="se")
                    so = psAo.tile([128, 2, 256], f32, tag="so", name="so")
                    t2e.append((se, ca))
                    t2o.append((so, cb))
                slot = i % 2
                mm(se[:, slot, :], k2[0:64, 128 * ja : 128 * ja + 128],
                   q2[0:64, qsl], True, True, tile_position=(0, 0))
                mm(so[:, slot, :], k2[64:128, 128 * jb : 128 * jb + 128],
                   q2[64:128, qsl], True, True, tile_position=(64, 0))
                if slot == 1 or i == half - 1:
                    width = slot + 1
                    for tile_, c0 in (t2e[-1], t2o[-1]):
                        nc.scalar.activation(
                            out=bexps[2][:, c0 : c0 + width, :],
                            in_=tile_[:, 0:width, :],
                            func=AFexp,
                        )
            for h in range(HPC):
                for lo, hi, base in mask_ops[cls]:
                    nc.vector.tensor_mul(
                        bexps[h][:, lo:hi, :],
                        bexps[h][:, lo:hi, :],
                        masks_sb[:, base : base + (hi - lo), :],
                    )
            return bexps

        def emit_pv_pair(blocks):
            """Transposed PV for 1-2 blocks: attnT [65, 256] per (block, head).
            Within a head the two blocks share 4 of their vall chunk
            stationaries, so iterate the chunk union once per head."""
            if not blocks:
                return
            ts = [t for t, _ in blocks]
            ranges = {t: _chunk_range(t) for t in ts}
            jlo = min(2 * t - 2 + ranges[t][0] for t in ts)
            jhi = max(2 * t - 2 + ranges[t][1] for t in ts)
            for h in range(HPC):
                ats = {}
                first = {}
                for t, _ in blocks:
                    ats[t] = psB.tile([HD + 1, 256], f32, tag="small", name="at")
                    first[t] = True
                for j in range(jlo, jhi):
                    for t, bexps in blocks:
                        c = j - (2 * t - 2)
                        cl, ch = ranges[t]
                        if cl <= c < ch:
                            mm(
                                ats[t],
                                vall[:, j, 2 * h, :],
                                bexps[h][:, c, :],
                                first[t],
                                False,
                            )
                            first[t] = False
                for t, _ in blocks:
                    qsl = slice(256 * t, 256 * (t + 1))
                    mm(ats[t], vall[0:G, 0, 2 * h, :], selexp[h][:, qsl], False, True)
                for t, _ in blocks:
                    osb = ob.tile([HD + 1, 256], f32, tag="osb", name="osb")
                    nc.vector.tensor_copy(out=osb, in_=ats[t])
                    nc.sync.dma_start(out=outT_d[h, t, :, :], in_=osb)

        # ================= fused main loop =================
        prev = []
        for st in range(NST + 1):
            if st < NST:
                if st == 0:
                    xt = xt0
                else:
                    xt = xpool.tile([128, 6, 512], bf16, tag="xt", name="xt")
                    nc.sync.dma_start(
                        out=xt, in_=xT[:, :, 512 * st : 512 * (st + 1)]
                    )
                emit_proj(st, xt)
            cur = []
            blocklist = (0,) if st == 0 else (2 * st - 1, 2 * st)
            for t in blocklist:
                if 0 <= t < NB:
                    cur.append((t, emit_qk(t)))
            if st == NST:
                # global-row PV rides here, overlapping the final band PVs
                ops3 = psB.tile([96, HD + 1], f32, tag="small", name="ops3")
                for c in range(NKC):
                    for h in range(HPC):
                        mm(
                            ops3[32 * h : 32 * h + G, :],
                            eg3[:, c, h, :],
                            vall[:, c, 2 * h + 1, :],
                            c == 0,
                            c == NKC - 1,
                            tile_position=(0, 32 * h),
                        )
                og = ob.tile([96, HD + 1], f32, tag="og", name="og")
                nc.vector.tensor_copy(out=og, in_=ops3)
                nc.sync.dma_start(out=outG_d[:], in_=og)
            emit_pv_pair(prev)
            prev = cur
        emit_pv_pair(prev)

    return nc


def _get_program():
    if "nc" not in _CACHE:
        nc = _build_program()
        nc.finalize()
        _CACHE["nc"] = nc
    return _CACHE["nc"]


def _prep_in_maps(hidden_states, Wq, bq, Wk, bk, Wv, bv, Wqg, bqg, Wkg, bkg, Wvg, bvg):
    hs = np.asarray(hidden_states, dtype=np.float32)
    f32 = np.float32
    in_maps = []
    for c in range(NCORES):
        b = c // 4
        cols = slice(HD * 3 * (c % 4), HD * (3 * (c % 4) + 3))

        wq = np.asarray(Wq)[:, cols] * SCALE     # [768, 192]
        wk = np.asarray(Wk)[:, cols]
        wkg = np.asarray(Wkg)[:, cols]
        wqgc = np.asarray(Wqg)[:, cols] * SCALE
        hcols = [slice(HD * h, HD * (h + 1)) for h in range(HPC)]
        # [q0|q1][k0|k1][q2|k2][kg0|kg1][kg2]
        wqkk = np.concatenate(
            [
                wq[:, hcols[0]], wq[:, hcols[1]],
                wk[:, hcols[0]], wk[:, hcols[1]],
                wq[:, hcols[2]], wk[:, hcols[2]],
                wkg[:, hcols[0]], wkg[:, hcols[1]],
                wkg[:, hcols[2]],
            ],
            axis=1,
        )
        wqgr = np.concatenate(
            [wqgc[:, hcols[0]], wqgc[:, hcols[1]], wqgc[:, hcols[2]]], axis=1
        )

        def seg(v, h, scale=1.0):
            return (np.asarray(v)[cols][HD * h : HD * (h + 1)] * scale).astype(f32)

        ball = np.zeros((128, 8), f32)
        ball[:, 0] = np.concatenate([seg(bq, 0, SCALE), seg(bq, 1, SCALE)])
        ball[:, 1] = np.concatenate([seg(bk, 0), seg(bk, 1)])
        ball[:, 2] = np.concatenate([seg(bq, 2, SCALE), np.zeros(64, f32)])
        ball[:, 3] = np.concatenate([seg(bkg, 0), seg(bkg, 1)])
        ball[:, 4] = np.concatenate([seg(bkg, 2), np.zeros(64, f32)])
        ball[:, 5] = np.concatenate([seg(bk, 2), np.zeros(64, f32)])
        ball[:, 6] = np.concatenate([seg(bqg, 0, SCALE), seg(bqg, 1, SCALE)])
        ball[:, 7] = np.concatenate([seg(bqg, 2, SCALE), np.zeros(64, f32)])

        bvvg = np.stack(
            [
                np.asarray(bv)[cols].reshape(HPC, HD),
                np.asarray(bvg)[cols].reshape(HPC, HD),
            ],
            axis=1,
        ).astype(f32)
        def tile3(a):
            # [768, W] -> [128, 6, W] with dm = c*128 + p -> [p, c, :]
            a = np.asarray(a)
            return np.ascontiguousarray(
                a.reshape(6, 128, a.shape[1]).transpose(1, 0, 2)
            ).astype(ml_dtypes.bfloat16)

        in_maps.append(
            {
                "xT": tile3(hs[b].T),
                "Wqkk": tile3(wqkk),
                "Wqg": tile3(wqgr),
                "Wvvg": tile3(
                    np.concatenate(
                        [np.asarray(Wv)[:, cols], np.asarray(Wvg)[:, cols]], axis=1
                    )
                ),
                "b_all": ball,
                "b_vvg": np.ascontiguousarray(
                    np.broadcast_to(bvvg[None], (128, HPC, 2, HD))
                ),
            }
        )
    return in_maps


def kernel(
    hidden_states,
    Wq,
    bq,
    Wk,
    bk,
    Wv,
    bv,
    Wqg,
    bqg,
    Wkg,
    bkg,
    Wvg,
    bvg,
    n_global,
):
    from concourse.bass_utils import run_bass_kernel_spmd

    assert int(n_global) == G
    nc = _get_program()
    in_maps = _prep_in_maps(
        hidden_states, Wq, bq, Wk, bk, Wv, bv, Wqg, bqg, Wkg, bkg, Wvg, bvg
    )
    res = run_bass_kernel_spmd(nc, in_maps, list(range(NCORES)))
    out = np.zeros((B, S, Dm), np.float32)
    for c in range(NCORES):
        b = c // 4
        base = HD * 3 * (c % 4)
        outT = res.results[c]["outT"]  # [3, NB, 65, 256]
        outG = res.results[c]["outG"]  # [96, 65]
        for h in range(HPC):
            oh = outT[h].transpose(1, 0, 2).reshape(HD + 1, S)
            att = oh[0:HD, :] / oh[HD : HD + 1, :]
            out[b, :, base + HD * h : base + HD * (h + 1)] = att.T
            og = outG[32 * h : 32 * h + G, 0:HD] / outG[32 * h : 32 * h + G, HD:]
            out[b, 0:G, base + HD * h : base + HD * (h + 1)] = og
    return out


# revision 54
# speedup vs baseline: 1.0011x; 1.0011x over previous
"""Longformer self-attention Trainium2 kernel (8-core SPMD), v3.

Sharding: core c handles batch b = c//4 and heads [3*(c%4), 3*(c%4)+3).
Each core computes its 3 heads' [4096, 64] outputs; the host divides by
the softmax denominator (exported as an extra row) and reassembles
[2, 4096, 768].

v3 design notes (on top of v2's fused projection/attention pipeline):
  - PE HAM clock gate: the PE runs at 1.2 GHz unless a ~3.4us activity
    window is busy; small-N matmul stretches re-throttle it.  v3 keeps
    every hot-loop matmul at N>=256:
      * PV is computed transposed: stationary v-chunk [128,65] (64 dims
        + ones column), moving bexp [128 kpos, 256 queries] -> attnT
        [65, 256] in PSUM, one matmul per kpos chunk (no half split).
        Row 64 is the softmax denominator; normalization happens on the
        HOST after the f32 attnT/denominator tile is DMA'd out.
      * QK (K=64 contraction) issues as concurrent row-tile pairs:
        heads 0/1 are packed at partitions 0-63/64-127 of shared q/k
        tiles, head 2 is duplicated into both halves via SBUF DMA and
        pairs its own chunks.  tile_position=(0,0)/(64,0); the pair's
        PSUM outputs come from disjoint even/odd pools (bank safety).
      * global-row PV col-tiles 3 heads at tile_position (0,0/32/64).
  - Weight columns are reordered per-core so each 128-wide PSUM
    d-chunk evacuates with a single DVE add: [q0|q1][k0|k1][q2|k2]
    [kg0|kg1][kg2].  Biases ride along as per-partition columns.
  - Masks: multiplicative 0/1 bf16 masks after exp (packed classes).
  - exp() without max subtraction (logits are O(1), safe in f32).
"""

import os
import sys

sys.path.insert(0, "/opt/trn_rl_repo")

import numpy as np
import ml_dtypes

B, S, Dm, H, WIN, G, HD = 2, 4096, 768, 12, 256, 16, 64
HPC = 3            # heads per core
NCORES = 8
DPC = HPC * HD     # 192 output dims per core
NB = S // WIN      # 16 query blocks
NKC = S // 128     # 32 kpos chunks of 128
NST = 8            # s-tiles of 512
SCALE = 1.0 / 8.0  # 1/sqrt(64)
QKK = 3 * DPC      # 576 merged q|k|kg output cols

_CACHE = {}


def _chunk_range(t):
    if t == 0:
        return 2, 6
    if t == NB - 1:
        return 0, 4
    return 0, 6


def _build_mask(t, c):
    p = np.arange(128)[:, None]
    r = np.arange(256)[None, :]
    kpos = (2 * t - 2 + c) * 128 + p
    i = 256 * t + r
    keep = (np.abs(kpos - i) <= WIN) & (kpos >= 0) & (kpos < S) & (kpos >= G)
    return keep.astype(np.float32)


def _mask_classes():
    """Packed mask chunks [nmask, 128, 256] and, per class, the list of
    (chunk_lo, chunk_hi, packed_idx) multiply ops."""
    packed = []

    def cls_of(t):
        return 0 if t == 0 else (1 if t == 1 else (3 if t == NB - 1 else 2))

    ops = {}
    for cls, trep in ((0, 0), (1, 1), (2, 7), (3, NB - 1)):
        cl, ch = _chunk_range(trep)
        need = [
            c for c in range(cl, ch) if not np.all(_build_mask(trep, c) == 1.0)
        ]
        groups = []
        for c in need:
            if groups and groups[-1][-1] == c - 1:
                groups[-1].append(c)
            else:
                groups.append([c])
        clsops = []
        for grp in groups:
            base = len(packed)
            for c in grp:
                packed.append(_build_mask(trep, c))
            clsops.append((grp[0], grp[-1] + 1, base))
        ops[cls] = clsops
    for t in range(NB):
        cls = cls_of(t)
        cl, ch = _chunk_range(t)
        covered = set()
        for lo, hi, base in ops[cls]:
            for c in range(lo, hi):
                assert np.array_equal(packed[base + (c - lo)], _build_mask(t, c))
                covered.add(c)
        for c in range(cl, ch):
            if c not in covered:
                assert np.all(_build_mask(t, c) == 1.0), (t, c)
    return np.stack(packed), ops, cls_of


def _patch_drain_and_barrier():
    """The walrus build in this container rejects >1 sync-wait on the CTRL
    (Drain) instruction that TileContext emits at exit.  Split the waits:
    keep one on the drain, emit the rest as explicit single-sem wait_ge
    instructions on the sync engine before the barrier."""
    import concourse.tile as tile
    from concourse import mybir
    from concourse.vector_clock import ScopedClock

    if getattr(tile.TileContext, "_ant_drain_patch", False):
        return

    def _drain_and_barrier(self, tick_clock, wait_clock):
        nc = self.nc
        drain_inst = nc.sync.drain()
        wait_clock.add_sem_waits(
            drain_inst.ins, ScopedClock({None: tick_clock.global_clock})
        )
        si = drain_inst.ins.sync_info
        waits = list(si.on_wait) if si is not None else []
        if len(waits) > 1:
            drain_inst.ins.sync_info = mybir.SyncInfo(
                on_wait=[waits[0]], on_update=list(si.on_update)
            )
            allocated = self.sems.allocated()
            by_name = {}
            for key, sem in allocated.items():
                by_name[str(key)] = sem
                nm = getattr(sem, "name", None)
                if nm is not None:
                    by_name[str(nm)] = sem
            for w in waits[1:]:
                sem = by_name[w.ant_name]
                nc.sync.wait_ge(sem, w.wait_value)
        nc.all_engine_barrier()
        assert self.sems is not None
        popped = nc._tile_sem_poison_stack.pop()
        assert popped is self._sem_poison
        nc.clear_and_free_semaphores(list(self.sems.allocated().values()))
        nc.all_engine_barrier()

    tile.TileContext._drain_and_barrier = _drain_and_barrier
    tile.TileContext._ant_drain_patch = True


def _build_program():
    import concourse.bass as bass
    import concourse.tile as tile
    from concourse import bacc, mybir

    _patch_drain_and_barrier()

    f32 = mybir.dt.float32
    bf16 = mybir.dt.bfloat16
    AF = mybir.ActivationFunctionType
    AFexp = AF.Exp

    nc = bacc.Bacc(None)

    xT = nc.dram_tensor("xT", [128, 6, S], bf16, kind="ExternalInput")
    # merged+reordered weight: [q0|q1][k0|k1][q2|k2][kg0|kg1][kg2]
    Wqkk = nc.dram_tensor("Wqkk", [128, 6, QKK], bf16, kind="ExternalInput")
    # qg weight reordered [qg0|qg1][qg2]
    Wqg = nc.dram_tensor("Wqg", [128, 6, DPC], bf16, kind="ExternalInput")
    Wvvg = nc.dram_tensor("Wvvg", [128, 6, 2 * DPC], bf16, kind="ExternalInput")
    # bias columns [128, 8]: col dc<5 = qkk evac bias for that d-chunk
    # (stacked per 64-row half); col 5 rows 0-63 = bk_h2; col 6 = qg01
    # stacked; col 7 rows 0-63 = bqg_h2
    b_all = nc.dram_tensor("b_all", [128, 8], f32, kind="ExternalInput")
    b_vvg = nc.dram_tensor("b_vvg", [128, HPC, 2, HD], f32, kind="ExternalInput")
    # unnormalized attn output (row 64 = softmax denominator)
    outT_d = nc.dram_tensor("outT", [HPC, NB, HD + 1, 256], f32, kind="ExternalOutput")
    outG_d = nc.dram_tensor("outG", [96, HD + 1], f32, kind="ExternalOutput")

    packed_np, mask_ops, tcls = _mask_classes()
    NMASK = packed_np.shape[0]
    masks_d = nc.inline_tensor(
        np.ascontiguousarray(packed_np.transpose(1, 0, 2)).astype(
            ml_dtypes.bfloat16
        ),
        name="masks",
    )

    from contextlib import ExitStack

    with tile.TileContext(nc) as tc, ExitStack() as ctx:
        const = ctx.enter_context(tc.tile_pool(name="const", bufs=1))
        ph = ctx.enter_context(tc.tile_pool(name="ph", bufs=1))
        xpool = ctx.enter_context(tc.tile_pool(name="xpool", bufs=2))
        bx = ctx.enter_context(tc.tile_pool(name="bx", bufs=12))
        ob = ctx.enter_context(tc.tile_pool(name="ob", bufs=4))
        # QK score pools: pair partners must land in different PSUM banks,
        # so base-0 tiles come from psAe and base-64 tiles from psAo.
        psAe = ctx.enter_context(tc.tile_pool(name="psAe", bufs=3, space="PSUM"))
        psAo = ctx.enter_context(tc.tile_pool(name="psAo", bufs=3, space="PSUM"))
        psB = ctx.enter_context(tc.tile_pool(name="psB", bufs=2, space="PSUM"))

        # startup loads: SP carries the critical path (wqkk-dc0 + x-tile 0,
        # then remaining weight strips); the Activation DGE carries the rest
        # in parallel (each dma_start costs ~650ns of issue time).
        wqkk = const.tile([128, 6, QKK], bf16, tag="wqkk", name="wqkk")
        xt0 = xpool.tile([128, 6, 512], bf16, tag="xt", name="xt")
        nc.sync.dma_start(out=wqkk[:, :, 0:128], in_=Wqkk[:, :, 0:128])
        nc.sync.dma_start(out=xt0, in_=xT[:, :, 0:512])
        for dcs in range(1, 5):
            e0 = 128 * dcs
            e1 = min(e0 + 128, QKK)
            nc.sync.dma_start(out=wqkk[:, :, e0:e1], in_=Wqkk[:, :, e0:e1])

        wvvg = const.tile([128, 6, 2 * DPC], bf16, tag="wvvg", name="wvvg")
        nc.scalar.dma_start(out=wvvg, in_=Wvvg[:])
        wqg = const.tile([128, 6, DPC], bf16, tag="wqg", name="wqg")
        nc.scalar.dma_start(out=wqg, in_=Wqg[:])
        ball = const.tile([128, 8], f32, tag="ball", name="ball")
        nc.scalar.dma_start(out=ball, in_=b_all[:])
        bvvg_sb = const.tile([128, HPC, 2, HD], f32, tag="bvvg", name="bvvg_sb")
        nc.scalar.dma_start(out=bvvg_sb, in_=b_vvg[:])
        masks_sb = const.tile([128, NMASK, 256], bf16, tag="masks", name="masks_sb")
        nc.scalar.dma_start(out=masks_sb, in_=masks_d[:])

        # PE warm-up: ~5us of dummy matmuls on never-written SBUF keep the
        # HAM activity window busy while the first DMAs land (the PE
        # otherwise starts at 1.2 GHz).  Results go to a scratch PSUM
        # tile that is never read.
        scrA = const.tile([128, 128], bf16, tag="scrA", name="scrA")
        scrB = const.tile([128, 512], bf16, tag="scrB", name="scrB")
        nc.gpsimd.memset(scrA[:], 0.5)
        nc.gpsimd.memset(scrB[:], 0.5)
        WARMUP = os.environ.get("KWARM", "0") == "1"
        if WARMUP:
            for wi in range(2):
                wps = psB.tile([128, 512], f32, tag="small", name="warm")
                for wj in range(12):
                    nc.tensor.matmul(wps, scrA, scrB, start=wj == 0, stop=wj == 11)

        # ---- persistent tensors ----
        # packed transposed projections: [0:64]=head A, [64:128]=head B
        qP = ph.tile([128, S], bf16, tag="qP", name="qP")     # q0|q1
        kP = ph.tile([128, S], bf16, tag="kP", name="kP")     # k0|k1
        kgP = ph.tile([128, S], bf16, tag="kgP", name="kgP")  # kg0|kg1
        q2 = ph.tile([128, S], bf16, tag="q2", name="q2")     # q2|q2(dup)
        k2 = ph.tile([128, S], bf16, tag="k2", name="k2")     # k2|k2(dup)
        kg2 = ph.tile([128, S], bf16, tag="kg2", name="kg2")  # kg2|kg2(dup)
        qgP = ph.tile([128, G], bf16, tag="qgP", name="qgP")  # qg0|qg1
        qg2 = ph.tile([128, G], bf16, tag="qg2", name="qg2")  # qg2|qg2(dup)

        def qhalf(h):  # (tile, part_lo) for q of head h
            return (qP, 0) if h == 0 else ((qP, 64) if h == 1 else (q2, 0))

        def khalf(h):
            return (kP, 0) if h == 0 else ((kP, 64) if h == 1 else (k2, 0))

        # v/vg interleaved with ones column: [:, chunk, 2h+0, :] = v head h,
        # [:, chunk, 2h+1, :] = vg head h ([:, :, :, 64] = 1.0)
        vall = ph.tile([128, NKC, 2 * HPC, HD + 1], bf16, tag="vall", name="vall")
        nc.vector.memset(vall[:, :, :, HD : HD + 1], 1.0)
        selexp = [
            ph.tile([G, S], bf16, tag=f"selexp{h}", name=f"selexp{h}")
            for h in range(HPC)
        ]
        eg3 = ph.tile([128, NKC, HPC, G], bf16, tag="eg3", name="eg3")

        def mm(out, lhsT, rhs, start, stop, tile_position=None):
            nc.tensor.matmul(
                out, lhsT, rhs, start=start, stop=stop, tile_position=tile_position
            )

        # qkk evac destinations per d-chunk: (dst_tile, rows, bias_col_ap)
        def emit_proj(st, xt):
            ssl = slice(512 * st, 512 * (st + 1))
            evac = [
                [(qP, None, 0)],
                [(kP, None, 1)],
                [(q2, (0, 64), 2), (k2, (64, 128), 5)],
                [(kgP, None, 3)],
                [(kg2, (0, 64), 4)],
            ]
            for dc in range(5):
                d0 = 128 * dc
                d1 = min(d0 + 128, QKK)
                ps = psB.tile([d1 - d0, 512], f32, tag="small", name="psqkk")
                for kc in range(6):
                    mm(ps, wqkk[:, kc, d0:d1], xt[:, kc, :], kc == 0, kc == 5)
                for dst, rows, bcol in evac[dc]:
                    if rows is None:
                        nc.vector.tensor_scalar_add(
                            dst[0 : d1 - d0, ssl], ps, ball[0 : d1 - d0, bcol : bcol + 1]
                        )
                    else:
                        r0, r1 = rows
                        nc.vector.tensor_scalar_add(
                            dst[0 : r1 - r0, ssl],
                            ps[r0:r1, :],
                            ball[0 : r1 - r0, bcol : bcol + 1],
                        )
            # duplicate head-2 halves: rows 0-63 -> rows 64-127 (QK pairing)
            for dst in (q2, k2):
                nc.sync.dma_start(out=dst[64:128, ssl], in_=dst[0:64, ssl])

            if st == 0:
                # qg: [128, 16] = qg0|qg1 stacked (full-M matmul), + qg2
                psq = psB.tile([128, G], f32, tag="small", name="psqg")
                for kc in range(6):
                    mm(psq, wqg[:, kc, 0:128], xt[:, kc, 0:G], kc == 0, kc == 5)
                nc.vector.tensor_scalar_add(qgP, psq, ball[:, 6:7])
                psq2 = psB.tile([64, G], f32, tag="small", name="psqg2")
                for kc in range(6):
                    mm(psq2, wqg[:, kc, 128:192], xt[:, kc, 0:G], kc == 0, kc == 5)
                nc.vector.tensor_scalar_add(qg2[0:64, :], psq2, ball[0:64, 7:8])

            # sel = q . k[:G] for this s-tile's queries; heads 0/1 paired
            sel01 = []
            for h in range(2):
                kt, p0 = khalf(h)
                qt, _ = qhalf(h)
                sps = psB.tile([G, 512], f32, tag="small", name="sps")
                mm(
                    sps,
                    kt[p0 : p0 + 64, 0:G],
                    qt[p0 : p0 + 64, ssl],
                    True,
                    True,
                    tile_position=(p0, 0),
                )
                sel01.append(sps)
            for h in range(2):
                nc.scalar.activation(
                    out=selexp[h][:, ssl], in_=sel01[h], func=AFexp
                )
            sps2 = psB.tile([G, 512], f32, tag="small", name="sps")
            mm(sps2, k2[0:64, 0:G], q2[0:64, ssl], True, True)
            nc.scalar.activation(out=selexp[2][:, ssl], in_=sps2, func=AFexp)

            # v/vg groups with the tiny glT matmuls interleaved BETWEEN
            # groups: an isolated cluster of N=16 matmuls starves the PE HAM
            # activity window and re-throttles the clock to 1.2 GHz for the
            # next ~3.4us.  One PSUM tile per glT head (adjacent-head glT
            # MMs use different PE row groups and execute concurrently --
            # their outputs must sit in different PSUM banks).
            gparams = (((kgP, 0), qgP), ((kgP, 64), qgP), ((kg2, 0), qg2))
            for sc in range(4):
                ci = 4 * st + sc
                msl = slice(128 * sc, 128 * (sc + 1))
                psv = psB.tile([128, 2 * DPC], f32, tag="small", name="psv")
                for kc in range(6):
                    mm(psv, xt[:, kc, msl], wvvg[:, kc, :], kc == 0, kc == 5)
                src = bass.AP(
                    tensor=psv.tensor,
                    offset=psv.offset,
                    ap=[psv.ap[0], [HD, HPC], [DPC, 2], [1, HD]],
                )
                dst = vall[:, ci, :, 0:HD].rearrange("p (h g) d -> p h g d", h=HPC)
                nc.vector.tensor_add(dst, src, bvvg_sb)
                if sc < 3:
                    (kgt, p0), qgt = gparams[sc]
                    gps = psB.tile([128, 4, G], f32, tag="small", name="gps")
                    for gsc in range(4):
                        gci = 4 * st + gsc
                        csl = slice(128 * gci, 128 * (gci + 1))
                        mm(
                            gps[:, gsc, :],
                            kgt[p0 : p0 + 64, csl],
                            qgt[p0 : p0 + 64, :],
                            True,
                            True,
                            tile_position=(p0, 0),
                        )
                    nc.scalar.activation(
                        out=eg3[:, 4 * st : 4 * st + 4, sc, :],
                        in_=gps,
                        func=AFexp,
                    )

        def emit_qk(t):
            """Paired QK scores + exp + mask for one block, 3 heads."""
            cl, ch = _chunk_range(t)
            n = ch - cl
            qsl = slice(256 * t, 256 * (t + 1))
            cls = tcls(t)
            bexps = [
                bx.tile([128, 6, 256], bf16, tag="bexp", name="bexp")
                for _ in range(HPC)
            ]

            # heads 0/1 pair on the same chunk (rows 0-63 vs 64-127); head 2
            # pairs chunk cl+i (base 0) with chunk cl+n/2+i (base 64, dup).
            # New [128,2,256] tile every 2 chunks.
            tiles01 = {0: [], 1: []}
            for ci, c in enumerate(range(cl, ch)):
                j = 2 * t - 2 + c
                jsl = slice(128 * j, 128 * (j + 1))
                if ci % 2 == 0:
                    se = psAe.tile([128, 2, 256], f32, tag="se", name="se")
                    so = psAo.tile([128, 2, 256], f32, tag="so", name="so")
                    tiles01[0].append((se, c))
                    tiles01[1].append((so, c))
                slot = ci % 2
                mm(se[:, slot, :], kP[0:64, jsl], qP[0:64, qsl], True, True,
                   tile_position=(0, 0))
                mm(so[:, slot, :], kP[64:128, jsl], qP[64:128, qsl], True, True,
                   tile_position=(64, 0))
                if slot == 1 or ci == n - 1:
                    width = slot + 1
                    for h, tl in ((0, tiles01[0][-1]), (1, tiles01[1][-1])):
                        tile_, c0 = tl
                        nc.scalar.activation(
                            out=bexps[h][:, c0 : c0 + width, :],
                            in_=tile_[:, 0:width, :],
                            func=AFexp,
                        )
            # head 2: pair (cl+i, cl+n/2+i) via the duplicated halves
            half = n // 2
            t2e, t2o = [], []
            for i in range(half):
                ca = cl + i
                cb = cl + half + i
                ja = 2 * t - 2 + ca
                jb = 2 * t - 2 + cb
                if i % 2 == 0:
                    se = psAe.tile([128, 2, 256], f32, tag="se", name="se")
                    so = psAo.tile([128, 2, 256], f32, tag="so", name="so")
                    t2e.append((se, ca))
                    t2o.append((so, cb))
                slot = i % 2
                mm(se[:, slot, :], k2[0:64, 128 * ja : 128 * ja + 128],
                   q2[0:64, qsl], True, True, tile_position=(0, 0))
                mm(so[:, slot, :], k2[64:128, 128 * jb : 128 * jb + 128],
                   q2[64:128, qsl], True, True, tile_position=(64, 0))
                if slot == 1 or i == half - 1:
                    width = slot + 1
                    for tile_, c0 in (t2e[-1], t2o[-1]):
                        nc.scalar.activation(
                            out=bexps[2][:, c0 : c0 + width, :],
                            in_=tile_[:, 0:width, :],
                            func=AFexp,
                        )
            for h in range(HPC):
                for lo, hi, base in mask_ops[cls]:
                    nc.vector.tensor_mul(
                        bexps[h][:, lo:hi, :],
                        bexps[h][:, lo:hi, :],
                        masks_sb[:, base : base + (hi - lo), :],
                    )
            return bexps

        def emit_pv_pair(blocks):
            """Transposed PV for 1-2 blocks: attnT [65, 256] per (block, head).
            Within a head the two blocks share 4 of their vall chunk
            stationaries, so iterate the chunk union once per head."""
            if not blocks:
                return
            ts = [t for t, _ in blocks]
            ranges = {t: _chunk_range(t) for t in ts}
            jlo = min(2 * t - 2 + ranges[t][0] for t in ts)
            jhi = max(2 * t - 2 + ranges[t][1] for t in ts)
            for h in range(HPC):
                ats = {}
                first = {}
                for t, _ in blocks:
                    ats[t] = psB.tile([HD + 1, 256], f32, tag="small", name="at")
                    first[t] = True
                for j in range(jlo, jhi):
                    for t, bexps in blocks:
                        c = j - (2 * t - 2)
                        cl, ch = ranges[t]
                        if cl <= c < ch:
                            mm(
                                ats[t],
                                vall[:, j, 2 * h, :],
                                bexps[h][:, c, :],
                                first[t],
                                False,
                            )
                            first[t] = False
                for t, _ in blocks:
                    qsl = slice(256 * t, 256 * (t + 1))
                    mm(ats[t], vall[0:G, 0, 2 * h, :], selexp[h][:, qsl], False, True)
                for t, _ in blocks:
                    osb = ob.tile([HD + 1, 256], f32, tag="osb", name="osb")
                    nc.vector.tensor_copy(out=osb, in_=ats[t])
                    nc.sync.dma_start(out=outT_d[h, t, :, :], in_=osb)

        # ================= fused main loop =================
        prev = []
        for st in range(NST + 1):
            if st < NST:
                if st == 0:
                    xt = xt0
                else:
                    xt = xpool.tile([128, 6, 512], bf16, tag="xt", name="xt")
                    nc.sync.dma_start(
                        out=xt, in_=xT[:, :, 512 * st : 512 * (st + 1)]
                    )
                emit_proj(st, xt)
            cur = []
            blocklist = [0] if st == 0 else [2 * st - 1, 2 * st]
            blocklist = [t for t in blocklist if 0 <= t < NB]
            pv_pending = list(prev)
            for t in blocklist:
                cur.append((t, emit_qk(t)))
                if pv_pending:
                    emit_pv_pair([pv_pending.pop(0)])
            if st == NST:
                # global-row PV rides here, overlapping the final band PVs
                ops3 = psB.tile([96, HD + 1], f32, tag="small", name="ops3")
                for c in range(NKC):
                    for h in range(HPC):
                        mm(
                            ops3[32 * h : 32 * h + G, :],
                            eg3[:, c, h, :],
                            vall[:, c, 2 * h + 1, :],
                            c == 0,
                            c == NKC - 1,
                            tile_position=(0, 32 * h),
                        )
                og = ob.tile([96, HD + 1], f32, tag="og", name="og")
                nc.vector.tensor_copy(out=og, in_=ops3)
                nc.sync.dma_start(out=outG_d[:], in_=og)
            emit_pv_pair(pv_pending)
            prev = cur
        emit_pv_pair(prev)

    return nc


def _get_program():
    if "nc" not in _CACHE:
        nc = _build_program()
        nc.finalize()
        _CACHE["nc"] = nc
    return _CACHE["nc"]


def _prep_in_maps(hidden_states, Wq, bq, Wk, bk, Wv, bv, Wqg, bqg, Wkg, bkg, Wvg, bvg):
    hs = np.asarray(hidden_states, dtype=np.float32)
    f32 = np.float32
    in_maps = []
    for c in range(NCORES):
        b = c // 4
        cols = slice(HD * 3 * (c % 4), HD * (3 * (c % 4) + 3))

        wq = np.asarray(Wq)[:, cols] * SCALE     # [768, 192]
        wk = np.asarray(Wk)[:, cols]
        wkg = np.asarray(Wkg)[:, cols]
        wqgc = np.asarray(Wqg)[:, cols] * SCALE
        hcols = [slice(HD * h, HD * (h + 1)) for h in range(HPC)]
        # [q0|q1][k0|k1][q2|k2][kg0|kg1][kg2]
        wqkk = np.concatenate(
            [
                wq[:, hcols[0]], wq[:, hcols[1]],
                wk[:, hcols[0]], wk[:, hcols[1]],
                wq[:, hcols[2]], wk[:, hcols[2]],
                wkg[:, hcols[0]], wkg[:, hcols[1]],
                wkg[:, hcols[2]],
            ],
            axis=1,
        )
        wqgr = np.concatenate(
            [wqgc[:, hcols[0]], wqgc[:, hcols[1]], wqgc[:, hcols[2]]], axis=1
        )

        def seg(v, h, scale=1.0):
            return (np.asarray(v)[cols][HD * h : HD * (h + 1)] * scale).astype(f32)

        ball = np.zeros((128, 8), f32)
        ball[:, 0] = np.concatenate([seg(bq, 0, SCALE), seg(bq, 1, SCALE)])
        ball[:, 1] = np.concatenate([seg(bk, 0), seg(bk, 1)])
        ball[:, 2] = np.concatenate([seg(bq, 2, SCALE), np.zeros(64, f32)])
        ball[:, 3] = np.concatenate([seg(bkg, 0), seg(bkg, 1)])
        ball[:, 4] = np.concatenate([seg(bkg, 2), np.zeros(64, f32)])
        ball[:, 5] = np.concatenate([seg(bk, 2), np.zeros(64, f32)])
        ball[:, 6] = np.concatenate([seg(bqg, 0, SCALE), seg(bqg, 1, SCALE)])
        ball[:, 7] = np.concatenate([seg(bqg, 2, SCALE), np.zeros(64, f32)])

        bvvg = np.stack(
            [
                np.asarray(bv)[cols].reshape(HPC, HD),
                np.asarray(bvg)[cols].reshape(HPC, HD),
            ],
            axis=1,
        ).astype(f32)
        def tile3(a):
            # [768, W] -> [128, 6, W] with dm = c*128 + p -> [p, c, :]
            a = np.asarray(a)
            return np.ascontiguousarray(
                a.reshape(6, 128, a.shape[1]).transpose(1, 0, 2)
            ).astype(ml_dtypes.bfloat16)

        in_maps.append(
            {
                "xT": tile3(hs[b].T),
                "Wqkk": tile3(wqkk),
                "Wqg": tile3(wqgr),
                "Wvvg": tile3(
                    np.concatenate(
                        [np.asarray(Wv)[:, cols], np.asarray(Wvg)[:, cols]], axis=1
                    )
                ),
                "b_all": ball,
                "b_vvg": np.ascontiguousarray(
                    np.broadcast_to(bvvg[None], (128, HPC, 2, HD))
                ),
            }
        )
    return in_maps


def kernel(
    hidden_states,
    Wq,
    bq,
    Wk,
    bk,
    Wv,
    bv,
    Wqg,
    bqg,
    Wkg,
    bkg,
    Wvg,
    bvg,
    n_global,
):
    from concourse.bass_utils import run_bass_kernel_spmd

    assert int(n_global) == G
    nc = _get_program()
    in_maps = _prep_in_maps(
        hidden_states, Wq, bq, Wk, bk, Wv, bv, Wqg, bqg, Wkg, bkg, Wvg, bvg
    )
    res = run_bass_kernel_spmd(nc, in_maps, list(range(NCORES)))
    out = np.zeros((B, S, Dm), np.float32)
    for c in range(NCORES):
        b = c // 4
        base = HD * 3 * (c % 4)
        outT = res.results[c]["outT"]  # [3, NB, 65, 256]
        outG = res.results[c]["outG"]  # [96, 65]
        for h in range(HPC):
            oh = outT[h].transpose(1, 0, 2).reshape(HD + 1, S)
            att = oh[0:HD, :] / oh[HD : HD + 1, :]
            out[b, :, base + HD * h : base + HD * (h + 1)] = att.T
            og = outG[32 * h : 32 * h + G, 0:HD] / outG[32 * h : 32 * h + G, HD:]
            out[b, 0:G, base + HD * h : base + HD * (h + 1)] = og
    return out


# revision 55
# speedup vs baseline: 1.0142x; 1.0130x over previous
"""Longformer self-attention Trainium2 kernel (8-core SPMD), v3.

Sharding: core c handles batch b = c//4 and heads [3*(c%4), 3*(c%4)+3).
Each core computes its 3 heads' [4096, 64] outputs; the host divides by
the softmax denominator (exported as an extra row) and reassembles
[2, 4096, 768].

v3 design notes (on top of v2's fused projection/attention pipeline):
  - PE HAM clock gate: the PE runs at 1.2 GHz unless a ~3.4us activity
    window is busy; small-N matmul stretches re-throttle it.  v3 keeps
    every hot-loop matmul at N>=256:
      * PV is computed transposed: stationary v-chunk [128,65] (64 dims
        + ones column), moving bexp [128 kpos, 256 queries] -> attnT
        [65, 256] in PSUM, one matmul per kpos chunk (no half split).
        Row 64 is the softmax denominator; normalization happens on the
        HOST after the f32 attnT/denominator tile is DMA'd out.
      * QK (K=64 contraction) issues as concurrent row-tile pairs:
        heads 0/1 are packed at partitions 0-63/64-127 of shared q/k
        tiles, head 2 is duplicated into both halves via SBUF DMA and
        pairs its own chunks.  tile_position=(0,0)/(64,0); the pair's
        PSUM outputs come from disjoint even/odd pools (bank safety).
      * global-row PV col-tiles 3 heads at tile_position (0,0/32/64).
  - Weight columns are reordered per-core so each 128-wide PSUM
    d-chunk evacuates with a single DVE add: [q0|q1][k0|k1][q2|k2]
    [kg0|kg1][kg2].  Biases ride along as per-partition columns.
  - Masks: multiplicative 0/1 bf16 masks after exp (packed classes).
  - exp() without max subtraction (logits are O(1), safe in f32).
"""

import os
import sys

sys.path.insert(0, "/opt/trn_rl_repo")

import numpy as np
import ml_dtypes

B, S, Dm, H, WIN, G, HD = 2, 4096, 768, 12, 256, 16, 64
HPC = 3            # heads per core
NCORES = 8
DPC = HPC * HD     # 192 output dims per core
NB = S // WIN      # 16 query blocks
NKC = S // 128     # 32 kpos chunks of 128
NST = 8            # s-tiles of 512
SCALE = 1.0 / 8.0  # 1/sqrt(64)
QKK = 3 * DPC      # 576 merged q|k|kg output cols

_CACHE = {}


def _chunk_range(t):
    if t == 0:
        return 2, 6
    if t == NB - 1:
        return 0, 4
    return 0, 6


def _build_mask(t, c):
    p = np.arange(128)[:, None]
    r = np.arange(256)[None, :]
    kpos = (2 * t - 2 + c) * 128 + p
    i = 256 * t + r
    keep = (np.abs(kpos - i) <= WIN) & (kpos >= 0) & (kpos < S) & (kpos >= G)
    return keep.astype(np.float32)


def _mask_classes():
    """Packed mask chunks [nmask, 128, 256] and, per class, the list of
    (chunk_lo, chunk_hi, packed_idx) multiply ops."""
    packed = []

    def cls_of(t):
        return 0 if t == 0 else (1 if t == 1 else (3 if t == NB - 1 else 2))

    ops = {}
    for cls, trep in ((0, 0), (1, 1), (2, 7), (3, NB - 1)):
        cl, ch = _chunk_range(trep)
        need = [
            c for c in range(cl, ch) if not np.all(_build_mask(trep, c) == 1.0)
        ]
        groups = []
        for c in need:
            if groups and groups[-1][-1] == c - 1:
                groups[-1].append(c)
            else:
                groups.append([c])
        clsops = []
        for grp in groups:
            base = len(packed)
            for c in grp:
                packed.append(_build_mask(trep, c))
            clsops.append((grp[0], grp[-1] + 1, base))
        ops[cls] = clsops
    for t in range(NB):
        cls = cls_of(t)
        cl, ch = _chunk_range(t)
        covered = set()
        for lo, hi, base in ops[cls]:
            for c in range(lo, hi):
                assert np.array_equal(packed[base + (c - lo)], _build_mask(t, c))
                covered.add(c)
        for c in range(cl, ch):
            if c not in covered:
                assert np.all(_build_mask(t, c) == 1.0), (t, c)
    return np.stack(packed), ops, cls_of


def _patch_drain_and_barrier():
    """The walrus build in this container rejects >1 sync-wait on the CTRL
    (Drain) instruction that TileContext emits at exit.  Split the waits:
    keep one on the drain, emit the rest as explicit single-sem wait_ge
    instructions on the sync engine before the barrier."""
    import concourse.tile as tile
    from concourse import mybir
    from concourse.vector_clock import ScopedClock

    if getattr(tile.TileContext, "_ant_drain_patch", False):
        return

    def _drain_and_barrier(self, tick_clock, wait_clock):
        nc = self.nc
        drain_inst = nc.sync.drain()
        wait_clock.add_sem_waits(
            drain_inst.ins, ScopedClock({None: tick_clock.global_clock})
        )
        si = drain_inst.ins.sync_info
        waits = list(si.on_wait) if si is not None else []
        if len(waits) > 1:
            drain_inst.ins.sync_info = mybir.SyncInfo(
                on_wait=[waits[0]], on_update=list(si.on_update)
            )
            allocated = self.sems.allocated()
            by_name = {}
            for key, sem in allocated.items():
                by_name[str(key)] = sem
                nm = getattr(sem, "name", None)
                if nm is not None:
                    by_name[str(nm)] = sem
            for w in waits[1:]:
                sem = by_name[w.ant_name]
                nc.sync.wait_ge(sem, w.wait_value)
        nc.all_engine_barrier()
        assert self.sems is not None
        popped = nc._tile_sem_poison_stack.pop()
        assert popped is self._sem_poison
        nc.clear_and_free_semaphores(list(self.sems.allocated().values()))
        nc.all_engine_barrier()

    tile.TileContext._drain_and_barrier = _drain_and_barrier
    tile.TileContext._ant_drain_patch = True


def _build_program():
    import concourse.bass as bass
    import concourse.tile as tile
    from concourse import bacc, mybir

    _patch_drain_and_barrier()

    f32 = mybir.dt.float32
    bf16 = mybir.dt.bfloat16
    AF = mybir.ActivationFunctionType
    AFexp = AF.Exp

    nc = bacc.Bacc(None)

    xT = nc.dram_tensor("xT", [128, 6, S], bf16, kind="ExternalInput")
    # merged+reordered weight: [q0|q1][k0|k1][q2|k2][kg0|kg1][kg2]
    Wqkk = nc.dram_tensor("Wqkk", [128, 6, QKK], bf16, kind="ExternalInput")
    # qg weight reordered [qg0|qg1][qg2]
    Wqg = nc.dram_tensor("Wqg", [128, 6, DPC], bf16, kind="ExternalInput")
    Wvvg = nc.dram_tensor("Wvvg", [128, 6, 2 * DPC], bf16, kind="ExternalInput")
    # bias columns [128, 8]: col dc<5 = qkk evac bias for that d-chunk
    # (stacked per 64-row half); col 5 rows 0-63 = bk_h2; col 6 = qg01
    # stacked; col 7 rows 0-63 = bqg_h2
    b_all = nc.dram_tensor("b_all", [128, 8], f32, kind="ExternalInput")
    b_vvg = nc.dram_tensor("b_vvg", [128, HPC, 2, HD], f32, kind="ExternalInput")
    # unnormalized attn output (row 64 = softmax denominator)
    outT_d = nc.dram_tensor("outT", [HPC, NB, HD + 1, 256], f32, kind="ExternalOutput")
    outG_d = nc.dram_tensor("outG", [96, HD + 1], f32, kind="ExternalOutput")

    packed_np, mask_ops, tcls = _mask_classes()
    NMASK = packed_np.shape[0]
    masks_d = nc.inline_tensor(
        np.ascontiguousarray(packed_np.transpose(1, 0, 2)).astype(
            ml_dtypes.bfloat16
        ),
        name="masks",
    )

    from contextlib import ExitStack

    with tile.TileContext(nc) as tc, ExitStack() as ctx:
        const = ctx.enter_context(tc.tile_pool(name="const", bufs=1))
        ph = ctx.enter_context(tc.tile_pool(name="ph", bufs=1))
        xpool = ctx.enter_context(tc.tile_pool(name="xpool", bufs=2))
        bx = ctx.enter_context(tc.tile_pool(name="bx", bufs=12))
        ob = ctx.enter_context(tc.tile_pool(name="ob", bufs=4))
        # QK score pools: pair partners must land in different PSUM banks,
        # so base-0 tiles come from psAe and base-64 tiles from psAo.
        psAe = ctx.enter_context(tc.tile_pool(name="psAe", bufs=3, space="PSUM"))
        psAo = ctx.enter_context(tc.tile_pool(name="psAo", bufs=3, space="PSUM"))
        psB = ctx.enter_context(tc.tile_pool(name="psB", bufs=2, space="PSUM"))

        # startup loads: SP carries the critical path (wqkk-dc0 + x-tile 0,
        # then remaining weight strips); the Activation DGE carries the rest
        # in parallel (each dma_start costs ~650ns of issue time).
        wqkk = const.tile([128, 6, QKK], bf16, tag="wqkk", name="wqkk")
        xt0 = xpool.tile([128, 6, 512], bf16, tag="xt", name="xt")
        nc.sync.dma_start(out=wqkk[:, :, 0:128], in_=Wqkk[:, :, 0:128])
        nc.sync.dma_start(out=xt0, in_=xT[:, :, 0:512])
        for dcs in range(1, 5):
            e0 = 128 * dcs
            e1 = min(e0 + 128, QKK)
            nc.sync.dma_start(out=wqkk[:, :, e0:e1], in_=Wqkk[:, :, e0:e1])

        wvvg = const.tile([128, 6, 2 * DPC], bf16, tag="wvvg", name="wvvg")
        nc.scalar.dma_start(out=wvvg, in_=Wvvg[:])
        wqg = const.tile([128, 6, DPC], bf16, tag="wqg", name="wqg")
        nc.scalar.dma_start(out=wqg, in_=Wqg[:])
        ball = const.tile([128, 8], f32, tag="ball", name="ball")
        nc.scalar.dma_start(out=ball, in_=b_all[:])
        bvvg_sb = const.tile([128, HPC, 2, HD], f32, tag="bvvg", name="bvvg_sb")
        nc.scalar.dma_start(out=bvvg_sb, in_=b_vvg[:])
        masks_sb = const.tile([128, NMASK, 256], bf16, tag="masks", name="masks_sb")
        nc.scalar.dma_start(out=masks_sb, in_=masks_d[:])

        # PE warm-up: ~5us of dummy matmuls on never-written SBUF keep the
        # HAM activity window busy while the first DMAs land (the PE
        # otherwise starts at 1.2 GHz).  Results go to a scratch PSUM
        # tile that is never read.
        scrA = const.tile([128, 128], bf16, tag="scrA", name="scrA")
        scrB = const.tile([128, 512], bf16, tag="scrB", name="scrB")
        nc.gpsimd.memset(scrA[:], 0.5)
        nc.gpsimd.memset(scrB[:], 0.5)
        WARMUP = os.environ.get("KWARM", "0") == "1"
        if WARMUP:
            for wi in range(2):
                wps = psB.tile([128, 512], f32, tag="small", name="warm")
                for wj in range(12):
                    nc.tensor.matmul(wps, scrA, scrB, start=wj == 0, stop=wj == 11)

        # ---- persistent tensors ----
        # packed transposed projections: [0:64]=head A, [64:128]=head B
        qP = ph.tile([128, S], bf16, tag="qP", name="qP")     # q0|q1
        kP = ph.tile([128, S], bf16, tag="kP", name="kP")     # k0|k1
        kgP = ph.tile([128, S], bf16, tag="kgP", name="kgP")  # kg0|kg1
        q2 = ph.tile([128, S], bf16, tag="q2", name="q2")     # q2|q2(dup)
        k2 = ph.tile([128, S], bf16, tag="k2", name="k2")     # k2|k2(dup)
        kg2 = ph.tile([128, S], bf16, tag="kg2", name="kg2")  # kg2|kg2(dup)
        qgP = ph.tile([128, G], bf16, tag="qgP", name="qgP")  # qg0|qg1
        qg2 = ph.tile([128, G], bf16, tag="qg2", name="qg2")  # qg2|qg2(dup)

        def qhalf(h):  # (tile, part_lo) for q of head h
            return (qP, 0) if h == 0 else ((qP, 64) if h == 1 else (q2, 0))

        def khalf(h):
            return (kP, 0) if h == 0 else ((kP, 64) if h == 1 else (k2, 0))

        # v/vg interleaved with ones column: [:, chunk, 2h+0, :] = v head h,
        # [:, chunk, 2h+1, :] = vg head h ([:, :, :, 64] = 1.0)
        vall = ph.tile([128, NKC, 2 * HPC, HD + 1], bf16, tag="vall", name="vall")
        nc.vector.memset(vall[:, :, :, HD : HD + 1], 1.0)
        selexp = [
            ph.tile([G, S], bf16, tag=f"selexp{h}", name=f"selexp{h}")
            for h in range(HPC)
        ]
        eg3 = ph.tile([128, NKC, HPC, G], bf16, tag="eg3", name="eg3")

        def mm(out, lhsT, rhs, start, stop, tile_position=None):
            nc.tensor.matmul(
                out, lhsT, rhs, start=start, stop=stop, tile_position=tile_position
            )

        # qkk evac destinations per d-chunk: (dst_tile, rows, bias_col_ap)
        def emit_proj(st, xt):
            ssl = slice(512 * st, 512 * (st + 1))
            evac = [
                [(qP, None, 0)],
                [(kP, None, 1)],
                [(q2, (0, 64), 2), (k2, (64, 128), 5)],
                [(kgP, None, 3)],
                [(kg2, (0, 64), 4)],
            ]
            for dc in range(5):
                d0 = 128 * dc
                d1 = min(d0 + 128, QKK)
                ps = psB.tile([d1 - d0, 512], f32, tag="small", name="psqkk")
                for kc in range(6):
                    mm(ps, wqkk[:, kc, d0:d1], xt[:, kc, :], kc == 0, kc == 5)
                for dst, rows, bcol in evac[dc]:
                    if rows is None:
                        nc.vector.tensor_scalar_add(
                            dst[0 : d1 - d0, ssl], ps, ball[0 : d1 - d0, bcol : bcol + 1]
                        )
                    else:
                        r0, r1 = rows
                        nc.vector.tensor_scalar_add(
                            dst[0 : r1 - r0, ssl],
                            ps[r0:r1, :],
                            ball[0 : r1 - r0, bcol : bcol + 1],
                        )
            # duplicate head-2 halves: rows 0-63 -> rows 64-127 (QK pairing)
            for dst in (q2, k2):
                nc.sync.dma_start(out=dst[64:128, ssl], in_=dst[0:64, ssl])

            if st == 0:
                # qg: [128, 16] = qg0|qg1 stacked (full-M matmul), + qg2
                psq = psB.tile([128, G], f32, tag="small", name="psqg")
                for kc in range(6):
                    mm(psq, wqg[:, kc, 0:128], xt[:, kc, 0:G], kc == 0, kc == 5)
                nc.vector.tensor_scalar_add(qgP, psq, ball[:, 6:7])
                psq2 = psB.tile([64, G], f32, tag="small", name="psqg2")
                for kc in range(6):
                    mm(psq2, wqg[:, kc, 128:192], xt[:, kc, 0:G], kc == 0, kc == 5)
                nc.vector.tensor_scalar_add(qg2[0:64, :], psq2, ball[0:64, 7:8])

            # sel = q . k[:G] for this s-tile's queries; heads 0/1 paired
            sel01 = []
            for h in range(2):
                kt, p0 = khalf(h)
                qt, _ = qhalf(h)
                sps = psB.tile([G, 512], f32, tag="small", name="sps")
                mm(
                    sps,
                    kt[p0 : p0 + 64, 0:G],
                    qt[p0 : p0 + 64, ssl],
                    True,
                    True,
                    tile_position=(p0, 0),
                )
                sel01.append(sps)
            for h in range(2):
                nc.scalar.activation(
                    out=selexp[h][:, ssl], in_=sel01[h], func=AFexp
                )
            sps2 = psB.tile([G, 512], f32, tag="small", name="sps")
            mm(sps2, k2[0:64, 0:G], q2[0:64, ssl], True, True)
            nc.scalar.activation(out=selexp[2][:, ssl], in_=sps2, func=AFexp)

            # v/vg groups with the tiny glT matmuls interleaved BETWEEN
            # groups: an isolated cluster of N=16 matmuls starves the PE HAM
            # activity window and re-throttles the clock to 1.2 GHz for the
            # next ~3.4us.  One PSUM tile per glT head (adjacent-head glT
            # MMs use different PE row groups and execute concurrently --
            # their outputs must sit in different PSUM banks).
            gparams = (((kgP, 0), qgP), ((kgP, 64), qgP), ((kg2, 0), qg2))
            for sc in range(4):
                ci = 4 * st + sc
                msl = slice(128 * sc, 128 * (sc + 1))
                psv = psB.tile([128, 2 * DPC], f32, tag="small", name="psv")
                for kc in range(6):
                    mm(psv, xt[:, kc, msl], wvvg[:, kc, :], kc == 0, kc == 5)
                src = bass.AP(
                    tensor=psv.tensor,
                    offset=psv.offset,
                    ap=[psv.ap[0], [HD, HPC], [DPC, 2], [1, HD]],
                )
                dst = vall[:, ci, :, 0:HD].rearrange("p (h g) d -> p h g d", h=HPC)
                nc.vector.tensor_add(dst, src, bvvg_sb)
                if sc < 3:
                    (kgt, p0), qgt = gparams[sc]
                    gps = psB.tile([128, 4, G], f32, tag="small", name="gps")
                    for gsc in range(4):
                        gci = 4 * st + gsc
                        csl = slice(128 * gci, 128 * (gci + 1))
                        mm(
                            gps[:, gsc, :],
                            kgt[p0 : p0 + 64, csl],
                            qgt[p0 : p0 + 64, :],
                            True,
                            True,
                            tile_position=(p0, 0),
                        )
                    nc.scalar.activation(
                        out=eg3[:, 4 * st : 4 * st + 4, sc, :],
                        in_=gps,
                        func=AFexp,
                    )

        def emit_qk(t):
            """Paired QK scores + exp + mask for one block, 3 heads."""
            cl, ch = _chunk_range(t)
            n = ch - cl
            qsl = slice(256 * t, 256 * (t + 1))
            cls = tcls(t)
            bexps = [
                bx.tile([128, 6, 256], bf16, tag="bexp", name="bexp")
                for _ in range(HPC)
            ]

            # heads 0/1 pair on the same chunk (rows 0-63 vs 64-127); head 2
            # pairs chunk cl+i (base 0) with chunk cl+n/2+i (base 64, dup).
            # New [128,2,256] tile every 2 chunks.
            tiles01 = {0: [], 1: []}
            for ci, c in enumerate(range(cl, ch)):
                j = 2 * t - 2 + c
                jsl = slice(128 * j, 128 * (j + 1))
                if ci % 2 == 0:
                    se = psAe.tile([128, 2, 256], f32, tag="se", name="se")
                    so = psAo.tile([128, 2, 256], f32, tag="so", name="so")
                    tiles01[0].append((se, c))
                    tiles01[1].append((so, c))
                slot = ci % 2
                mm(se[:, slot, :], kP[0:64, jsl], qP[0:64, qsl], True, True,
                   tile_position=(0, 0))
                mm(so[:, slot, :], kP[64:128, jsl], qP[64:128, qsl], True, True,
                   tile_position=(64, 0))
                if slot == 1 or ci == n - 1:
                    width = slot + 1
                    for h, tl in ((0, tiles01[0][-1]), (1, tiles01[1][-1])):
                        tile_, c0 = tl
                        nc.scalar.activation(
                            out=bexps[h][:, c0 : c0 + width, :],
                            in_=tile_[:, 0:width, :],
                            func=AFexp,
                        )
            # head 2: pair (cl+i, cl+n/2+i) via the duplicated halves
            half = n // 2
            t2e, t2o = [], []
            for i in range(half):
                ca = cl + i
                cb = cl + half + i
                ja = 2 * t - 2 + ca
                jb = 2 * t - 2 + cb
                if i % 2 == 0:
                    se = psAe.tile([128, 2, 256], f32, tag="se", name="se")
                    so = psAo.tile([128, 2, 256], f32, tag="so", name="so")
                    t2e.append((se, ca))
                    t2o.append((so, cb))
                slot = i % 2
                mm(se[:, slot, :], k2[0:64, 128 * ja : 128 * ja + 128],
                   q2[0:64, qsl], True, True, tile_position=(0, 0))
                mm(so[:, slot, :], k2[64:128, 128 * jb : 128 * jb + 128],
                   q2[64:128, qsl], True, True, tile_position=(64, 0))
                if slot == 1 or i == half - 1:
                    width = slot + 1
                    for tile_, c0 in (t2e[-1], t2o[-1]):
                        nc.scalar.activation(
                            out=bexps[2][:, c0 : c0 + width, :],
                            in_=tile_[:, 0:width, :],
                            func=AFexp,
                        )
            for h in range(HPC):
                for lo, hi, base in mask_ops[cls]:
                    nc.vector.tensor_mul(
                        bexps[h][:, lo:hi, :],
                        bexps[h][:, lo:hi, :],
                        masks_sb[:, base : base + (hi - lo), :],
                    )
            return bexps

        def emit_pv_pair(blocks):
            """Transposed PV for 1-2 blocks: attnT [65, 256] per (block, head).
            Within a head the two blocks share 4 of their vall chunk
            stationaries, so iterate the chunk union once per head."""
            if not blocks:
                return
            ts = [t for t, _ in blocks]
            ranges = {t: _chunk_range(t) for t in ts}
            jlo = min(2 * t - 2 + ranges[t][0] for t in ts)
            jhi = max(2 * t - 2 + ranges[t][1] for t in ts)
            # lhsT is widened from 65 to 128 columns (spilling into the
            # adjacent vg columns): a 128-col stationary qualifies for FWL
            # and its LDWEIGHTS overlaps the previous matmul; at 65 cols the
            # load serializes (~+90ns per matmul).  Output rows 65..127 are
            # garbage and simply not copied out.
            for h in range(HPC):
                ats = {}
                first = {}
                for t, _ in blocks:
                    ats[t] = psB.tile([128, 256], f32, tag="small", name="at")
                    first[t] = True
                for j in range(jlo, jhi):
                    vj = vall[:, j, :, :].rearrange("p a b -> p (a b)")
                    for t, bexps in blocks:
                        c = j - (2 * t - 2)
                        cl, ch = ranges[t]
                        if cl <= c < ch:
                            mm(
                                ats[t],
                                vj[:, 130 * h : 130 * h + 128],
                                bexps[h][:, c, :],
                                first[t],
                                False,
                            )
                            first[t] = False
                v0 = vall[0:G, 0, :, :].rearrange("p a b -> p (a b)")
                for t, _ in blocks:
                    qsl = slice(256 * t, 256 * (t + 1))
                    mm(
                        ats[t],
                        v0[:, 130 * h : 130 * h + 128],
                        selexp[h][:, qsl],
                        False,
                        True,
                    )
                for t, _ in blocks:
                    osb = ob.tile([HD + 1, 256], f32, tag="osb", name="osb")
                    nc.vector.tensor_copy(out=osb, in_=ats[t][0 : HD + 1, :])
                    nc.sync.dma_start(out=outT_d[h, t, :, :], in_=osb)

        # ================= fused main loop =================
        prev = []
        for st in range(NST + 1):
            if st < NST:
                if st == 0:
                    xt = xt0
                else:
                    xt = xpool.tile([128, 6, 512], bf16, tag="xt", name="xt")
                    nc.sync.dma_start(
                        out=xt, in_=xT[:, :, 512 * st : 512 * (st + 1)]
                    )
                emit_proj(st, xt)
            cur = []
            blocklist = [0] if st == 0 else [2 * st - 1, 2 * st]
            blocklist = [t for t in blocklist if 0 <= t < NB]
            pv_pending = list(prev)
            for t in blocklist:
                cur.append((t, emit_qk(t)))
                if pv_pending:
                    emit_pv_pair([pv_pending.pop(0)])
            if st == NST:
                # global-row PV rides here, overlapping the final band PVs
                ops3 = psB.tile([96, HD + 1], f32, tag="small", name="ops3")
                for c in range(NKC):
                    for h in range(HPC):
                        mm(
                            ops3[32 * h : 32 * h + G, :],
                            eg3[:, c, h, :],
                            vall[:, c, 2 * h + 1, :],
                            c == 0,
                            c == NKC - 1,
                            tile_position=(0, 32 * h),
                        )
                og = ob.tile([96, HD + 1], f32, tag="og", name="og")
                nc.vector.tensor_copy(out=og, in_=ops3)
                nc.sync.dma_start(out=outG_d[:], in_=og)
            emit_pv_pair(pv_pending)
            prev = cur
        emit_pv_pair(prev)

    return nc


def _get_program():
    if "nc" not in _CACHE:
        nc = _build_program()
        nc.finalize()
        _CACHE["nc"] = nc
    return _CACHE["nc"]


def _prep_in_maps(hidden_states, Wq, bq, Wk, bk, Wv, bv, Wqg, bqg, Wkg, bkg, Wvg, bvg):
    hs = np.asarray(hidden_states, dtype=np.float32)
    f32 = np.float32
    in_maps = []
    for c in range(NCORES):
        b = c // 4
        cols = slice(HD * 3 * (c % 4), HD * (3 * (c % 4) + 3))

        wq = np.asarray(Wq)[:, cols] * SCALE     # [768, 192]
        wk = np.asarray(Wk)[:, cols]
        wkg = np.asarray(Wkg)[:, cols]
        wqgc = np.asarray(Wqg)[:, cols] * SCALE
        hcols = [slice(HD * h, HD * (h + 1)) for h in range(HPC)]
        # [q0|q1][k0|k1][q2|k2][kg0|kg1][kg2]
        wqkk = np.concatenate(
            [
                wq[:, hcols[0]], wq[:, hcols[1]],
                wk[:, hcols[0]], wk[:, hcols[1]],
                wq[:, hcols[2]], wk[:, hcols[2]],
                wkg[:, hcols[0]], wkg[:, hcols[1]],
                wkg[:, hcols[2]],
            ],
            axis=1,
        )
        wqgr = np.concatenate(
            [wqgc[:, hcols[0]], wqgc[:, hcols[1]], wqgc[:, hcols[2]]], axis=1
        )

        def seg(v, h, scale=1.0):
            return (np.asarray(v)[cols][HD * h : HD * (h + 1)] * scale).astype(f32)

        ball = np.zeros((128, 8), f32)
        ball[:, 0] = np.concatenate([seg(bq, 0, SCALE), seg(bq, 1, SCALE)])
        ball[:, 1] = np.concatenate([seg(bk, 0), seg(bk, 1)])
        ball[:, 2] = np.concatenate([seg(bq, 2, SCALE), np.zeros(64, f32)])
        ball[:, 3] = np.concatenate([seg(bkg, 0), seg(bkg, 1)])
        ball[:, 4] = np.concatenate([seg(bkg, 2), np.zeros(64, f32)])
        ball[:, 5] = np.concatenate([seg(bk, 2), np.zeros(64, f32)])
        ball[:, 6] = np.concatenate([seg(bqg, 0, SCALE), seg(bqg, 1, SCALE)])
        ball[:, 7] = np.concatenate([seg(bqg, 2, SCALE), np.zeros(64, f32)])

        bvvg = np.stack(
            [
                np.asarray(bv)[cols].reshape(HPC, HD),
                np.asarray(bvg)[cols].reshape(HPC, HD),
            ],
            axis=1,
        ).astype(f32)
        def tile3(a):
            # [768, W] -> [128, 6, W] with dm = c*128 + p -> [p, c, :]
            a = np.asarray(a)
            return np.ascontiguousarray(
                a.reshape(6, 128, a.shape[1]).transpose(1, 0, 2)
            ).astype(ml_dtypes.bfloat16)

        in_maps.append(
            {
                "xT": tile3(hs[b].T),
                "Wqkk": tile3(wqkk),
                "Wqg": tile3(wqgr),
                "Wvvg": tile3(
                    np.concatenate(
                        [np.asarray(Wv)[:, cols], np.asarray(Wvg)[:, cols]], axis=1
                    )
                ),
                "b_all": ball,
                "b_vvg": np.ascontiguousarray(
                    np.broadcast_to(bvvg[None], (128, HPC, 2, HD))
                ),
            }
        )
    return in_maps


def kernel(
    hidden_states,
    Wq,
    bq,
    Wk,
    bk,
    Wv,
    bv,
    Wqg,
    bqg,
    Wkg,
    bkg,
    Wvg,
    bvg,
    n_global,
):
    from concourse.bass_utils import run_bass_kernel_spmd

    assert int(n_global) == G
    nc = _get_program()
    in_maps = _prep_in_maps(
        hidden_states, Wq, bq, Wk, bk, Wv, bv, Wqg, bqg, Wkg, bkg, Wvg, bvg
    )
    res = run_bass_kernel_spmd(nc, in_maps, list(range(NCORES)))
    out = np.zeros((B, S, Dm), np.float32)
    for c in range(NCORES):
        b = c // 4
        base = HD * 3 * (c % 4)
        outT = res.results[c]["outT"]  # [3, NB, 65, 256]
        outG = res.results[c]["outG"]  # [96, 65]
        for h in range(HPC):
            oh = outT[h].transpose(1, 0, 2).reshape(HD + 1, S)
            att = oh[0:HD, :] / oh[HD : HD + 1, :]
            out[b, :, base + HD * h : base + HD * (h + 1)] = att.T
            og = outG[32 * h : 32 * h + G, 0:HD] / outG[32 * h : 32 * h + G, HD:]
            out[b, 0:G, base + HD * h : base + HD * (h + 1)] = og
    return out
